# revision 14
# baseline (speedup 1.0000x reference)
"""BinaryPathEncoder Trainium2 kernel.

maps[p] = prod over the binary path of p (LSB-first, dropping the leading 1)
of trans[bit], where trans = [expm(h0).T, expm(h1).T, I], h_b = P_b - P_b^T.

Algorithm (per core, data-parallel over 128 positions):
  - expm via scaling (1/2^6) + degree-15 Taylor (Paterson-Stockmeyer) + 6
    squarings, computing the pair (e^X, e^-X) so squarings need no transposes.
  - Binary-path product tables F_j[m] = prod of j bits of m (LSB-first),
    j=1..5, both plain (FTAB, for moving operands) and transposed (TABT, for
    stationary operands).  maps[p] = F5[p&31] @ (F5[(p>>5)&31] @ T6[p>>10])
    with T6/identity fallbacks for short paths -> exactly 2 matmuls/position.
  - Per-position matmuls are grouped by stationary table entry (static lhsT
    slice); moving operands are gathered with dynamic (register) offsets fed
    from per-core index tensors.  Group capacities are padded to the max
    across cores (single SPMD program); pad lanes are discarded on the host.

The host computes index tensors only (bit twiddling); all FLOPs run on
device.  Output is assembled (pad-dropping + unpermute) on the host.
"""

import numpy as np

import concourse.bass as bass
import concourse.mybir as mybir
import concourse.tile as tile
from concourse import bacc
from concourse.bass_utils import run_bass_kernel_spmd
from concourse.masks import make_identity

P = 128
NCORES = 8
NPOS = 128  # positions per core
S_SCALE = 6  # squarings; ||A|| ~ 37 -> ||X|| ~ 0.58
NLEVELS = 5  # table levels (5 bits per chunk)
DUMP_SLOT = NPOS  # Wstage slot for pad writes
FP = mybir.dt.float32
BF = mybir.dt.bfloat16

# table entry bases: level j block starts at 1 + 2*(2^(j-1)-1), entries 2^j
TB = {j: 1 + 2 * (2 ** (j - 1) - 1) for j in range(1, NLEVELS + 1)}
NTAB = 1 + 2 * (2**NLEVELS - 1)  # 63 entries


def _tab_idx(q):
    """Table index of F_{bl(q)-1}[q - 2^j] == path product of q (q >= 2)."""
    j = int(q).bit_length() - 1
    return TB[j] + (q - (1 << j))


def _t6(q):
    return 0 if q < 2 else _tab_idx(q)


def _f5(m):
    return TB[NLEVELS] + m


def _plan(unique):
    """Balanced core assignment, group structure, offsets, output mapping."""
    unique = np.asarray(unique).astype(np.int64)
    N = unique.shape[0]
    # balance step2 groups (key l = p&31) exactly across cores; within each
    # l-bucket greedily balance step1 keys (m = (p>>5)&31)
    by_l = {}
    for g, p in enumerate(unique):
        by_l.setdefault(int(p) & 31, []).append(g)
    core_members = [[] for _ in range(NCORES)]
    cnt_l = np.zeros((NCORES, 32), np.int32)
    cnt_m = np.zeros((NCORES, 64), np.int32)
    rot = 0
    for l, idxs in sorted(by_l.items()):
        nl = len(idxs)
        room = [nl // NCORES + (1 if (c - rot) % NCORES < nl % NCORES else 0)
                for c in range(NCORES)]
        rot += nl % NCORES
        for g in idxs:
            p = int(unique[g])
            m = (p >> 5) & 31 if p >= 1024 else 32 + (p & 31)  # step1 key proxy
            best, bc = None, None
            for c in range(NCORES):
                if room[c] <= 0 or len(core_members[c]) >= NPOS:
                    continue
                score = (cnt_m[c, m], len(core_members[c]))
                if best is None or score < best:
                    best, bc = score, c
            if bc is None:
                bc = min(range(NCORES), key=lambda c: len(core_members[c]))
            else:
                room[bc] -= 1
            cnt_m[bc, m] += 1
            core_members[bc].append(g)
    for c in range(NCORES):
        assert len(core_members[c]) == NPOS, [len(x) for x in core_members]

    # swap refinement: exchange same-l positions between cores to flatten the
    # per-core step1-key histograms (reduces padded group capacities)
    def e1key(p):
        return ((p >> 5) & 31) if p >= 1024 else 64 + (p & 31)

    core_of = {}
    for c in range(NCORES):
        for g in core_members[c]:
            core_of[g] = c
    cnt = np.zeros((NCORES, 128), np.int32)
    for c in range(NCORES):
        for g in core_members[c]:
            cnt[c, e1key(int(unique[g]))] += 1

    rng = np.random.default_rng(12345)
    gl = list(range(N))
    for _ in range(4):
        improved = False
        for l, idxs in by_l.items():
            for a in range(len(idxs)):
                for b in range(a + 1, len(idxs)):
                    g1, g2 = idxs[a], idxs[b]
                    c1, c2 = core_of[g1], core_of[g2]
                    if c1 == c2:
                        continue
                    m1 = e1key(int(unique[g1]))
                    m2 = e1key(int(unique[g2]))
                    if m1 == m2:
                        continue
                    before = (max(cnt[:, m1]) + max(cnt[:, m2]))
                    cnt[c1, m1] -= 1; cnt[c2, m1] += 1
                    cnt[c2, m2] -= 1; cnt[c1, m2] += 1
                    after = (max(cnt[:, m1]) + max(cnt[:, m2]))
                    if after < before:
                        core_of[g1], core_of[g2] = c2, c1
                        improved = True
                    else:
                        cnt[c1, m1] += 1; cnt[c2, m1] -= 1
                        cnt[c2, m2] += 1; cnt[c1, m2] -= 1
        if not improved:
            break
    core_members = [[] for _ in range(NCORES)]
    for g in range(N):
        core_members[core_of[g]].append(g)
    for c in range(NCORES):
        assert len(core_members[c]) == NPOS

    tasks1 = [[] for _ in range(NCORES)]  # (e1, rhs_entry, global idx)
    tasks2 = [[] for _ in range(NCORES)]  # (e2, global idx)
    for c in range(NCORES):
        for g in core_members[c]:
            p = int(unique[g])
            h, hh = p >> 5, p >> 10
            if p >= 32:
                if h >= 32:
                    e1, r1 = _f5(h & 31), _t6(hh)
                else:
                    e1, r1 = _t6(h), 0
                e2 = _f5(p & 31)
            else:
                e1, r1 = 0, 0
                e2 = _t6(p)
            tasks1[c].append((e1, r1, g))
            tasks2[c].append((e2, g))

    def group(tasks):
        keys = sorted({t[0] for c in range(NCORES) for t in tasks[c]})
        percore = []
        caps = {e: 0 for e in keys}
        for c in range(NCORES):
            d = {e: [] for e in keys}
            for t in tasks[c]:
                d[t[0]].append(t)
            percore.append(d)
            for e in keys:
                caps[e] = max(caps[e], len(d[e]))
        return keys, caps, percore

    k1, cap1, pc1 = group(tasks1)
    k2, cap2, pc2 = group(tasks2)
    groups1 = [(e, cap1[e]) for e in k1]
    groups2 = [(e, cap2[e]) for e in k2]
    S1 = sum(c for _, c in groups1)
    S2 = sum(c for _, c in groups2)

    # step2 lane numbering: each group's range padded to a multiple of 4 so
    # 4-lane chunks never straddle groups.  S2P = padded total.
    S2P = sum(((cap + 3) // 4) * 4 for _, cap in groups2)
    lane2 = [dict() for _ in range(NCORES)]
    outmap = [dict() for _ in range(NCORES)]  # padded step2 lane -> global idx
    for c in range(NCORES):
        k = 0
        for e, cap in groups2:
            lst = pc2[c][e]
            for j in range(len(lst)):
                _, g = lst[j]
                lane2[c][g] = k + j
                outmap[c][k + j] = g
            k += ((cap + 3) // 4) * 4

    # every step2 lane slot (incl. pads) must be written by some step1 lane so
    # the wide step2 matmuls never read uninitialized SBUF.  Pad step1 lanes
    # cover the step2 pad slots; if a core runs short, extra lanes are added.
    uncovered = []
    for c in range(NCORES):
        real = set(lane2[c].values())
        uncovered.append([L for L in range(S2P) if L not in real])
    n_pad1 = [S1 - NPOS] * NCORES  # pad step1 lanes per core
    extra = max(0, max(len(uncovered[c]) - n_pad1[c] for c in range(NCORES)))
    if extra:
        e0, cap0 = groups1[-1]
        groups1 = groups1[:-1] + [(e0, cap0 + extra)]
        S1 += extra

    offs1_rhs = np.zeros((NCORES, 1, S1), np.int32)
    offs1_wdst = np.full((NCORES, 1, S1), S2P * P, np.int32)  # dump slot = S2P
    for c in range(NCORES):
        cov = list(uncovered[c])
        k = 0
        for e, cap in groups1:
            lst = pc1[c][e]
            for j in range(cap):
                if j < len(lst):
                    _, r1, g = lst[j]
                    offs1_rhs[c, 0, k] = r1 * P
                    offs1_wdst[c, 0, k] = lane2[c][g] * P
                elif cov:
                    offs1_rhs[c, 0, k] = 0
                    offs1_wdst[c, 0, k] = cov.pop() * P
                k += 1
        assert not cov, f"core {c}: {len(cov)} step2 slots uncovered"
    return groups1, groups2, S1, S2P, offs1_rhs, offs1_wdst, outmap


# degree-15 Taylor coefficients 1/k!
_FACT = [1.0]
for _k in range(1, 16):
    _FACT.append(_FACT[-1] * _k)
_C = [1.0 / f for f in _FACT]

PE = mybir.EngineType.PE
DVE = mybir.EngineType.DVE
ADD = mybir.AluOpType.add
SUB = mybir.AluOpType.subtract
MULT = mybir.AluOpType.mult


def _build(nc, groups1, groups2, S1, S2):
    prims_d = nc.dram_tensor("prims", [2, P, P], FP, kind="ExternalInput").ap()
    o1r_d = nc.dram_tensor("offs1_rhs", [1, S1], mybir.dt.int32, kind="ExternalInput").ap()
    o1w_d = nc.dram_tensor("offs1_wdst", [1, S1], mybir.dt.int32, kind="ExternalInput").ap()
    nch = (S2 + 3) // 4
    raw_d = nc.dram_tensor("rawout", [nch, P, 4 * P], FP, kind="ExternalOutput").ap()

    with tile.TileContext(nc) as tc:
        with tc.tile_pool(name="persist", bufs=1) as pp:
            ftab_h = pp.tile([P, NTAB * P], BF, name="ftab_h")
            ftab_l = pp.tile([P, NTAB * P], BF, name="ftab_l")
            tabt_h = pp.tile([P, NTAB * P], BF, name="tabt_h")
            tabt_l = pp.tile([P, NTAB * P], BF, name="tabt_l")
            eh = pp.tile([P, 2 * P], BF, name="eh")  # expP0 | expP1 (hi)
            el = pp.tile([P, 2 * P], BF, name="el")
            ident = pp.tile([P, P], FP, name="ident")
            ptile = pp.tile([P, 2 * P], FP, name="ptile")
            o1r = pp.tile([1, S1], mybir.dt.int32, name="o1r")
            o1w = pp.tile([1, S1], mybir.dt.int32, name="o1w")

            nc.sync.dma_start(out=ptile[:], in_=prims_d.rearrange("b r c -> r b c"))
            nc.sync.dma_start(out=o1r[:], in_=o1r_d[:])
            nc.sync.dma_start(out=o1w[:], in_=o1w_d[:])
            make_identity(nc, ident[:])

            # W staging in step2-lane order (+1 dump slot).  Every slot <= S2
            # is written by some step1 lane (the host covers pad slots), so no
            # zero-fill is needed; only the dump slot needs initialization.
            wstage_h = pp.tile([P, (S2 + 1) * P], BF, name="wstage_h")
            wstage_l = pp.tile([P, (S2 + 1) * P], BF, name="wstage_l")
            nc.vector.memset(wstage_h[:, S2 * P :], 0.0)
            nc.vector.memset(wstage_l[:, S2 * P :], 0.0)

            # ---------------- expm ----------------
            with (
                tc.tile_pool(name="expm_sb", bufs=1) as ep,
                tc.tile_pool(name="expm_ps", bufs=4, space="PSUM") as eps,
            ):
                sc = 1.0 / (1 << S_SCALE)
                xt = ep.tile([P, 2 * P], FP, name="xt")
                nxt = ep.tile([P, 2 * P], FP, name="nxt")
                x2 = ep.tile([P, 2 * P], FP, name="x2")
                x3 = ep.tile([P, 2 * P], FP, name="x3")
                x4 = ep.tile([P, 2 * P], FP, name="x4")

                def bsl(t, b):
                    return t[:, b * P : (b + 1) * P]

                for b in range(2):
                    tp = eps.tile([P, 4 * P], FP, space="PSUM", name="tp", tag="eps")[:, :P]
                    nc.tensor.transpose(tp[:], bsl(ptile, b), ident[:])
                    # X = (P - P^T)/2^s ; NX = -X
                    nc.vector.scalar_tensor_tensor(
                        out=bsl(xt, b), in0=bsl(ptile, b), scalar=1.0, in1=tp[:],
                        op0=MULT, op1=SUB,
                    )
                for b in range(2):
                    nc.vector.tensor_scalar_mul(bsl(nxt, b), bsl(xt, b), -sc)
                for b in range(2):
                    nc.vector.tensor_scalar_mul(bsl(xt, b), bsl(xt, b), sc)
                for b in range(2):
                    ps = eps.tile([P, 4 * P], FP, space="PSUM", name="ps", tag="eps")[:, :P]
                    nc.tensor.matmul(ps[:], lhsT=bsl(nxt, b), rhs=bsl(xt, b), start=True, stop=True)
                    nc.vector.tensor_copy(out=bsl(x2, b), in_=ps[:])
                for b in range(2):
                    ps = eps.tile([P, 4 * P], FP, space="PSUM", name="ps", tag="eps")[:, :P]
                    nc.tensor.matmul(ps[:], lhsT=bsl(x2, b), rhs=bsl(xt, b), start=True, stop=True)
                    nc.vector.tensor_copy(out=bsl(x3, b), in_=ps[:])
                for b in range(2):
                    ps = eps.tile([P, 4 * P], FP, space="PSUM", name="ps", tag="eps")[:, :P]
                    nc.tensor.matmul(ps[:], lhsT=bsl(x2, b), rhs=bsl(x2, b), start=True, stop=True)
                    nc.vector.tensor_copy(out=bsl(x4, b), in_=ps[:])

                # B_i = c0 I + c1 X + c2 X2 + c3 X3 ; Bm_i with -X, -X3
                bp = [[None] * 2 for _ in range(4)]
                bm = [[None] * 2 for _ in range(4)]
                for i in range(4):
                    c0, c1, c2, c3 = _C[4 * i : 4 * i + 4]
                    ic = ep.tile([P, P], FP, name=f"ic{i}")
                    nc.vector.tensor_scalar_mul(ic[:], ident[:], c0)
                    for b in range(2):
                        tmp = ep.tile([P, P], FP, name=f"tmp{i}{b}")
                        nc.vector.tensor_scalar_mul(tmp[:], bsl(xt, b), c1)
                        vo = ep.tile([P, P], FP, name=f"vo{i}{b}")
                        nc.vector.scalar_tensor_tensor(
                            out=vo[:], in0=bsl(x3, b), scalar=c3, in1=tmp[:],
                            op0=MULT, op1=ADD,
                        )
                        ve = ep.tile([P, P], FP, name=f"ve{i}{b}")
                        nc.vector.scalar_tensor_tensor(
                            out=ve[:], in0=bsl(x2, b), scalar=c2, in1=ic[:],
                            op0=MULT, op1=ADD,
                        )
                        tb_ = ep.tile([P, P], FP, name=f"tb{i}{b}")
                        nc.vector.tensor_tensor(out=tb_[:], in0=ve[:], in1=vo[:], op=ADD)
                        tm_ = ep.tile([P, P], FP, name=f"tm{i}{b}")
                        nc.vector.tensor_tensor(out=tm_[:], in0=ve[:], in1=vo[:], op=SUB)
                        bp[i][b] = tb_
                        bm[i][b] = tm_

                # Paterson-Stockmeyer chains -> S = e^X, ST = e^-X (scaled)
                s_t, st_t = [None, None], [None, None]
                for b in range(2):
                    for which, bs, dst in ((0, bp, s_t), (1, bm, st_t)):
                        z = bs[3][b]
                        for i in (2, 1, 0):
                            ps = eps.tile([P, 4 * P], FP, space="PSUM", name="psz", tag="eps")[:, :P]
                            nc.tensor.matmul(ps[:], lhsT=bsl(x4, b), rhs=z[:], start=True, stop=True)
                            znew = ep.tile([P, P], FP, name=f"z{which}{b}{i}")
                            nc.vector.tensor_tensor(out=znew[:], in0=ps[:], in1=bs[i][b][:], op=ADD)
                            z = znew
                        dst[b] = z

                for lvl in range(S_SCALE):
                    for b in range(2):
                        ps1 = eps.tile([P, 4 * P], FP, space="PSUM", name="psq1", tag="eps")[:, :P]
                        nc.tensor.matmul(ps1[:], lhsT=st_t[b][:], rhs=s_t[b][:], start=True, stop=True)
                        ps2 = eps.tile([P, 4 * P], FP, space="PSUM", name="psq2", tag="eps")[:, :P]
                        nc.tensor.matmul(ps2[:], lhsT=s_t[b][:], rhs=st_t[b][:], start=True, stop=True)
                        sn = ep.tile([P, P], FP, name=f"s{lvl}{b}")
                        stn = ep.tile([P, P], FP, name=f"st{lvl}{b}")
                        nc.vector.tensor_copy(out=sn[:], in_=ps1[:])
                        nc.vector.tensor_copy(out=stn[:], in_=ps2[:])
                        s_t[b], st_t[b] = sn, stn

                # final: E = expP (S), tables level 1 (hi/lo bf16 pairs)
                def slot(t, e):
                    return t[:, e * P : (e + 1) * P]

                def wpair(th, tl, dst_h, dst_l, src):
                    nc.scalar.copy(out=dst_h, in_=src)
                    nc.vector.tensor_tensor(out=dst_l, in0=src, in1=dst_h, op=SUB)

                def wpair_e(th, tl, e, src):
                    wpair(th, tl, slot(th, e), slot(tl, e), src)

                for b in range(2):
                    wpair(eh, el, bsl(eh, b), bsl(el, b), s_t[b][:])
                    wpair_e(ftab_h, ftab_l, TB[1] + b, st_t[b][:])
                    wpair_e(tabt_h, tabt_l, TB[1] + b, s_t[b][:])
                wpair_e(ftab_h, ftab_l, 0, ident[:])
                wpair_e(tabt_h, tabt_l, 0, ident[:])

                # ---------------- tables j=2..5 ----------------
                for j in range(2, NLEVELS + 1):
                    half = 1 << (j - 1)
                    # plain: F_j[2t+b] = M_b @ F_{j-1}[t] ; out = E_b^T... lhsT=E_b
                    for b in range(2):
                        for cs in range(0, half, 4):
                            cn = min(4, half - cs)
                            ps = eps.tile([P, 4 * P], FP, space="PSUM", name="pstab", tag="eps")
                            rsl = slice((TB[j - 1] + cs) * P, (TB[j - 1] + cs + cn) * P)
                            nc.tensor.matmul(
                                ps[:, : cn * P], lhsT=bsl(eh, b), rhs=ftab_h[:, rsl],
                                start=True, stop=False,
                            )
                            nc.tensor.matmul(
                                ps[:, : cn * P], lhsT=bsl(eh, b), rhs=ftab_l[:, rsl],
                                start=False, stop=False,
                            )
                            nc.tensor.matmul(
                                ps[:, : cn * P], lhsT=bsl(el, b), rhs=ftab_h[:, rsl],
                                start=False, stop=True,
                            )
                            # interleaved write: entries TB[j] + 2*(cs+t) + b
                            for t in range(cn):
                                ei = TB[j] + 2 * (cs + t) + b
                                wpair_e(ftab_h, ftab_l, ei, ps[:, t * P : (t + 1) * P])
                    # transposed: TABT[TB_j + 2t + b] = F_{j-1}[t]^T @ expP_b
                    for t in range(half):
                        ps = eps.tile([P, 4 * P], FP, space="PSUM", name="pstt", tag="eps")[:, :2 * P]
                        nc.tensor.matmul(
                            ps[:], lhsT=slot(ftab_h, TB[j - 1] + t), rhs=eh[:, 0 : 2 * P],
                            start=True, stop=False,
                        )
                        nc.tensor.matmul(
                            ps[:], lhsT=slot(ftab_h, TB[j - 1] + t), rhs=el[:, 0 : 2 * P],
                            start=False, stop=False,
                        )
                        nc.tensor.matmul(
                            ps[:], lhsT=slot(ftab_l, TB[j - 1] + t), rhs=eh[:, 0 : 2 * P],
                            start=False, stop=True,
                        )
                        for tt2 in range(2):
                            wpair_e(
                                tabt_h, tabt_l, TB[j] + 2 * t + tt2,
                                ps[:, tt2 * P : (tt2 + 1) * P],
                            )

            # ---------------- per-position stage ----------------
            with (
                tc.tile_pool(name="ps1", bufs=5, space="PSUM") as ps1p,
                tc.tile_pool(name="ps2", bufs=3, space="PSUM") as ps2p,
                tc.tile_pool(name="outp", bufs=8) as outp,
            ):
                k = 0
                for e, cap in groups1:
                    lhs_h = tabt_h[:, e * P : (e + 1) * P]
                    lhs_l = tabt_l[:, e * P : (e + 1) * P]
                    for _ in range(cap):
                        rv = nc.values_load(
                            o1r[0:1, k : k + 1], engines={PE},
                            min_val=0, max_val=(NTAB - 1) * P,
                            skip_runtime_bounds_check=True,
                        )
                        ps = ps1p.tile([P, P], FP, space="PSUM", name="ps1t")
                        nc.tensor.matmul(
                            ps[:], lhsT=lhs_h, rhs=ftab_h[:, bass.ds(rv, P)],
                            start=True, stop=False,
                        )
                        nc.tensor.matmul(
                            ps[:], lhsT=lhs_h, rhs=ftab_l[:, bass.ds(rv, P)],
                            start=False, stop=False,
                        )
                        nc.tensor.matmul(
                            ps[:], lhsT=lhs_l, rhs=ftab_h[:, bass.ds(rv, P)],
                            start=False, stop=True,
                        )
                        rwa = nc.values_load(
                            o1w[0:1, k : k + 1], engines={mybir.EngineType.Activation},
                            min_val=0, max_val=S2 * P,
                            skip_runtime_bounds_check=True,
                        )
                        rwv = nc.values_load(
                            o1w[0:1, k : k + 1], engines={DVE},
                            min_val=0, max_val=S2 * P,
                            skip_runtime_bounds_check=True,
                        )
                        nc.scalar.copy(out=wstage_h[:, bass.ds(rwa, P)], in_=ps[:])
                        nc.vector.tensor_tensor(
                            out=wstage_l[:, bass.ds(rwv, P)], in0=ps[:],
                            in1=wstage_h[:, bass.ds(rwv, P)], op=SUB,
                        )
                        k += 1

                # step2: static wide matmuls over 4-lane chunks (lane
                # numbering is group-padded to x4; wstage pre-zeroed so the
                # full 4-lane width is always valid)
                k = 0
                for e, cap in groups2:
                    lhs_h = tabt_h[:, e * P : (e + 1) * P]
                    lhs_l = tabt_l[:, e * P : (e + 1) * P]
                    capp = ((cap + 3) // 4) * 4
                    for cs in range(0, capp, 4):
                        lane0 = k + cs
                        rsl_h = wstage_h[:, lane0 * P : (lane0 + 4) * P]
                        rsl_l = wstage_l[:, lane0 * P : (lane0 + 4) * P]
                        ps = ps2p.tile([P, 4 * P], FP, space="PSUM", name="ps2t")
                        nc.tensor.matmul(ps[:], lhsT=lhs_h, rhs=rsl_h, start=True, stop=False)
                        nc.tensor.matmul(ps[:], lhsT=lhs_h, rhs=rsl_l, start=False, stop=False)
                        nc.tensor.matmul(ps[:], lhsT=lhs_l, rhs=rsl_h, start=False, stop=True)
                        ch = outp.tile([P, 4 * P], FP, name="outchunk")
                        if (lane0 // 4) % 3 != 2:
                            nc.vector.tensor_copy(out=ch[:], in_=ps[:])
                        else:
                            nc.scalar.copy(out=ch[:], in_=ps[:])
                        nc.sync.dma_start(out=raw_d[lane0 // 4], in_=ch[:])
                    k += capp
    return nc


def _run(nc, in_maps, trace=False):
    return run_bass_kernel_spmd(nc, in_maps, list(range(NCORES)), trace=trace)


def build_all(unique, primitives):
    """Build program + per-core inputs. Returns (nc, in_maps, assemble)."""
    unique = np.asarray(unique).astype(np.int64)
    primitives = np.ascontiguousarray(np.asarray(primitives, dtype=np.float32))
    groups1, groups2, S1, S2P, o1r, o1w, outmap = _plan(unique)

    nc = bacc.Bacc("TRN2", target_bir_lowering=False, debug=False)
    _build(nc, groups1, groups2, S1, S2P)
    nc.compile()

    in_maps = [
        {
            "prims": primitives,
            "offs1_rhs": o1r[c],
            "offs1_wdst": o1w[c],
        }
        for c in range(NCORES)
    ]

    def assemble(results):
        full = np.empty((NCORES * NPOS, P, P), np.float32)
        for c in range(NCORES):
            raw = results[c]["rawout"]  # [nch, P, 4P]
            for lane, g in outmap[c].items():
                full[g] = raw[lane // 4][:, (lane % 4) * P : (lane % 4 + 1) * P]
        return full

    return nc, in_maps, assemble


def kernel(unique, primitives):
    nc, in_maps, assemble = build_all(unique, primitives)
    res = _run(nc, in_maps, trace=False)
    return assemble(res.results)


# revision 16
# speedup vs baseline: 1.0556x; 1.0556x over previous
"""BinaryPathEncoder Trainium2 kernel.

maps[p] = prod over the binary path of p (LSB-first, dropping the leading 1)
of trans[bit], where trans = [expm(h0).T, expm(h1).T, I], h_b = P_b - P_b^T.

Algorithm (per core, data-parallel over 128 positions):
  - expm via scaling (1/2^6) + degree-15 Taylor (Paterson-Stockmeyer) + 6
    squarings, computing the pair (e^X, e^-X) so squarings need no transposes.
  - Binary-path product tables F_j[m] = prod of j bits of m (LSB-first),
    j=1..5, both plain (FTAB, for moving operands) and transposed (TABT, for
    stationary operands).  maps[p] = F5[p&31] @ (F5[(p>>5)&31] @ T6[p>>10])
    with T6/identity fallbacks for short paths -> exactly 2 matmuls/position.
  - Per-position matmuls are grouped by stationary table entry (static lhsT
    slice); moving operands are gathered with dynamic (register) offsets fed
    from per-core index tensors.  Group capacities are padded to the max
    across cores (single SPMD program); pad lanes are discarded on the host.

The host computes index tensors only (bit twiddling); all FLOPs run on
device.  Output is assembled (pad-dropping + unpermute) on the host.
"""

import numpy as np

import concourse.bass as bass
import concourse.mybir as mybir
import concourse.tile as tile
from concourse import bacc
from concourse.bass_utils import run_bass_kernel_spmd
from concourse.masks import make_identity

P = 128
NCORES = 8
NPOS = 128  # positions per core
S_SCALE = 6  # squarings; ||A|| ~ 37 -> ||X|| ~ 0.58
NLEVELS = 5  # table levels (5 bits per chunk)
DUMP_SLOT = NPOS  # Wstage slot for pad writes
FP = mybir.dt.float32
BF = mybir.dt.bfloat16

# table entry bases: level j block starts at 1 + 2*(2^(j-1)-1), entries 2^j
TB = {j: 1 + 2 * (2 ** (j - 1) - 1) for j in range(1, NLEVELS + 1)}
NTAB = 1 + 2 * (2**NLEVELS - 1)  # 63 entries


def _tab_idx(q):
    """Table index of F_{bl(q)-1}[q - 2^j] == path product of q (q >= 2)."""
    j = int(q).bit_length() - 1
    return TB[j] + (q - (1 << j))


def _t6(q):
    return 0 if q < 2 else _tab_idx(q)


def _f5(m):
    return TB[NLEVELS] + m


def _plan(unique):
    """Balanced core assignment, group structure, offsets, output mapping."""
    unique = np.asarray(unique).astype(np.int64)
    N = unique.shape[0]
    # balance step2 groups (key l = p&31) exactly across cores; within each
    # l-bucket greedily balance step1 keys (m = (p>>5)&31)
    by_l = {}
    for g, p in enumerate(unique):
        by_l.setdefault(int(p) & 31, []).append(g)
    core_members = [[] for _ in range(NCORES)]
    cnt_l = np.zeros((NCORES, 32), np.int32)
    cnt_m = np.zeros((NCORES, 64), np.int32)
    rot = 0
    for l, idxs in sorted(by_l.items()):
        nl = len(idxs)
        room = [nl // NCORES + (1 if (c - rot) % NCORES < nl % NCORES else 0)
                for c in range(NCORES)]
        rot += nl % NCORES
        for g in idxs:
            p = int(unique[g])
            m = (p >> 5) & 31 if p >= 1024 else 32 + (p & 31)  # step1 key proxy
            best, bc = None, None
            for c in range(NCORES):
                if room[c] <= 0 or len(core_members[c]) >= NPOS:
                    continue
                score = (cnt_m[c, m], len(core_members[c]))
                if best is None or score < best:
                    best, bc = score, c
            if bc is None:
                bc = min(range(NCORES), key=lambda c: len(core_members[c]))
            else:
                room[bc] -= 1
            cnt_m[bc, m] += 1
            core_members[bc].append(g)
    for c in range(NCORES):
        assert len(core_members[c]) == NPOS, [len(x) for x in core_members]

    # swap refinement: exchange same-l positions between cores to flatten the
    # per-core step1-key histograms (reduces padded group capacities)
    def e1key(p):
        return ((p >> 5) & 31) if p >= 1024 else 64 + (p & 31)

    core_of = {}
    for c in range(NCORES):
        for g in core_members[c]:
            core_of[g] = c
    cnt = np.zeros((NCORES, 128), np.int32)
    for c in range(NCORES):
        for g in core_members[c]:
            cnt[c, e1key(int(unique[g]))] += 1

    rng = np.random.default_rng(12345)
    gl = list(range(N))
    for _ in range(4):
        improved = False
        for l, idxs in by_l.items():
            for a in range(len(idxs)):
                for b in range(a + 1, len(idxs)):
                    g1, g2 = idxs[a], idxs[b]
                    c1, c2 = core_of[g1], core_of[g2]
                    if c1 == c2:
                        continue
                    m1 = e1key(int(unique[g1]))
                    m2 = e1key(int(unique[g2]))
                    if m1 == m2:
                        continue
                    before = (max(cnt[:, m1]) + max(cnt[:, m2]))
                    cnt[c1, m1] -= 1; cnt[c2, m1] += 1
                    cnt[c2, m2] -= 1; cnt[c1, m2] += 1
                    after = (max(cnt[:, m1]) + max(cnt[:, m2]))
                    if after < before:
                        core_of[g1], core_of[g2] = c2, c1
                        improved = True
                    else:
                        cnt[c1, m1] += 1; cnt[c2, m1] -= 1
                        cnt[c2, m2] += 1; cnt[c1, m2] -= 1
        if not improved:
            break
    core_members = [[] for _ in range(NCORES)]
    for g in range(N):
        core_members[core_of[g]].append(g)
    for c in range(NCORES):
        assert len(core_members[c]) == NPOS

    tasks1 = [[] for _ in range(NCORES)]  # (e1, rhs_entry, global idx)
    tasks2 = [[] for _ in range(NCORES)]  # (e2, global idx)
    for c in range(NCORES):
        for g in core_members[c]:
            p = int(unique[g])
            h, hh = p >> 5, p >> 10
            if p >= 32:
                if h >= 32:
                    e1, r1 = _f5(h & 31), _t6(hh)
                else:
                    e1, r1 = _t6(h), 0
                e2 = _f5(p & 31)
            else:
                e1, r1 = 0, 0
                e2 = _t6(p)
            tasks1[c].append((e1, r1, g))
            tasks2[c].append((e2, g))

    def group(tasks):
        keys = sorted({t[0] for c in range(NCORES) for t in tasks[c]})
        percore = []
        caps = {e: 0 for e in keys}
        for c in range(NCORES):
            d = {e: [] for e in keys}
            for t in tasks[c]:
                d[t[0]].append(t)
            percore.append(d)
            for e in keys:
                caps[e] = max(caps[e], len(d[e]))
        return keys, caps, percore

    k1, cap1, pc1 = group(tasks1)
    k2, cap2, pc2 = group(tasks2)
    groups1 = [(e, cap1[e]) for e in k1]
    groups2 = [(e, cap2[e]) for e in k2]
    S1 = sum(c for _, c in groups1)
    S2 = sum(c for _, c in groups2)

    # step2 lane numbering: each group's range padded to a multiple of 4 so
    # 4-lane chunks never straddle groups.  S2P = padded total.
    S2P = sum(((cap + 3) // 4) * 4 for _, cap in groups2)
    lane2 = [dict() for _ in range(NCORES)]
    outmap = [dict() for _ in range(NCORES)]  # padded step2 lane -> global idx
    for c in range(NCORES):
        k = 0
        for e, cap in groups2:
            lst = pc2[c][e]
            for j in range(len(lst)):
                _, g = lst[j]
                lane2[c][g] = k + j
                outmap[c][k + j] = g
            k += ((cap + 3) // 4) * 4

    # every step2 lane slot (incl. pads) must be written by some step1 lane so
    # the wide step2 matmuls never read uninitialized SBUF.  Pad step1 lanes
    # cover the step2 pad slots; if a core runs short, extra lanes are added.
    uncovered = []
    for c in range(NCORES):
        real = set(lane2[c].values())
        uncovered.append([L for L in range(S2P) if L not in real])
    n_pad1 = [S1 - NPOS] * NCORES  # pad step1 lanes per core
    extra = max(0, max(len(uncovered[c]) - n_pad1[c] for c in range(NCORES)))
    if extra:
        e0, cap0 = groups1[-1]
        groups1 = groups1[:-1] + [(e0, cap0 + extra)]
        S1 += extra

    offs1_rhs = np.zeros((NCORES, 1, S1), np.int32)
    offs1_wdst = np.full((NCORES, 1, S1), S2P * P, np.int32)  # dump slot = S2P
    for c in range(NCORES):
        cov = list(uncovered[c])
        k = 0
        for e, cap in groups1:
            lst = pc1[c][e]
            for j in range(cap):
                if j < len(lst):
                    _, r1, g = lst[j]
                    offs1_rhs[c, 0, k] = r1 * P
                    offs1_wdst[c, 0, k] = lane2[c][g] * P
                elif cov:
                    offs1_rhs[c, 0, k] = 0
                    offs1_wdst[c, 0, k] = cov.pop() * P
                k += 1
        assert not cov, f"core {c}: {len(cov)} step2 slots uncovered"
    used_t = sorted({e for e, _ in groups1} | {e for e, _ in groups2})
    return groups1, groups2, S1, S2P, offs1_rhs, offs1_wdst, outmap, used_t


# degree-15 Taylor coefficients 1/k!
_FACT = [1.0]
for _k in range(1, 16):
    _FACT.append(_FACT[-1] * _k)
_C = [1.0 / f for f in _FACT]

PE = mybir.EngineType.PE
DVE = mybir.EngineType.DVE
ADD = mybir.AluOpType.add
SUB = mybir.AluOpType.subtract
MULT = mybir.AluOpType.mult


def _build(nc, groups1, groups2, S1, S2, used_t=None):
    prims_d = nc.dram_tensor("prims", [2, P, P], FP, kind="ExternalInput").ap()
    o1r_d = nc.dram_tensor("offs1_rhs", [1, S1], mybir.dt.int32, kind="ExternalInput").ap()
    o1w_d = nc.dram_tensor("offs1_wdst", [1, S1], mybir.dt.int32, kind="ExternalInput").ap()
    nch = (S2 + 3) // 4
    raw_d = nc.dram_tensor("rawout", [nch, P, 4 * P], FP, kind="ExternalOutput").ap()

    with tile.TileContext(nc) as tc:
        with tc.tile_pool(name="persist", bufs=1) as pp:
            ftab_h = pp.tile([P, NTAB * P], BF, name="ftab_h")
            ftab_l = pp.tile([P, NTAB * P], BF, name="ftab_l")
            tabt_h = pp.tile([P, NTAB * P], BF, name="tabt_h")
            tabt_l = pp.tile([P, NTAB * P], BF, name="tabt_l")
            ftab_h3 = None  # set below (3-d strided views)
            ftab_l3 = None
            eh = pp.tile([P, 2 * P], BF, name="eh")  # expP0 | expP1 (hi)
            el = pp.tile([P, 2 * P], BF, name="el")
            ident = pp.tile([P, P], FP, name="ident")
            ptile = pp.tile([P, 2 * P], FP, name="ptile")
            o1r = pp.tile([1, S1], mybir.dt.int32, name="o1r")
            o1w = pp.tile([1, S1], mybir.dt.int32, name="o1w")

            ftab_h3 = ftab_h[:].rearrange("p (e c) -> p e c", c=P)
            ftab_l3 = ftab_l[:].rearrange("p (e c) -> p e c", c=P)
            nc.sync.dma_start(out=ptile[:], in_=prims_d.rearrange("b r c -> r b c"))
            nc.sync.dma_start(out=o1r[:], in_=o1r_d[:])
            nc.sync.dma_start(out=o1w[:], in_=o1w_d[:])
            make_identity(nc, ident[:])

            # W staging in step2-lane order (+1 dump slot).  Every slot <= S2
            # is written by some step1 lane (the host covers pad slots), so no
            # zero-fill is needed; only the dump slot needs initialization.
            wstage_h = pp.tile([P, (S2 + 1) * P], BF, name="wstage_h")
            wstage_l = pp.tile([P, (S2 + 1) * P], BF, name="wstage_l")
            nc.vector.memset(wstage_h[:, S2 * P :], 0.0)
            nc.vector.memset(wstage_l[:, S2 * P :], 0.0)

            # ---------------- expm ----------------
            with (
                tc.tile_pool(name="expm_sb", bufs=1) as ep,
                tc.tile_pool(name="expm_ps", bufs=4, space="PSUM") as eps,
            ):
                sc = 1.0 / (1 << S_SCALE)
                xt = ep.tile([P, 2 * P], FP, name="xt")
                nxt = ep.tile([P, 2 * P], FP, name="nxt")
                x2 = ep.tile([P, 2 * P], FP, name="x2")
                x3 = ep.tile([P, 2 * P], FP, name="x3")
                x4 = ep.tile([P, 2 * P], FP, name="x4")

                def bsl(t, b):
                    return t[:, b * P : (b + 1) * P]

                for b in range(2):
                    tp = eps.tile([P, 4 * P], FP, space="PSUM", name="tp", tag="eps")[:, :P]
                    nc.tensor.transpose(tp[:], bsl(ptile, b), ident[:])
                    # X = (P - P^T)/2^s ; NX = -X
                    nc.vector.scalar_tensor_tensor(
                        out=bsl(xt, b), in0=bsl(ptile, b), scalar=1.0, in1=tp[:],
                        op0=MULT, op1=SUB,
                    )
                for b in range(2):
                    nc.vector.tensor_scalar_mul(bsl(nxt, b), bsl(xt, b), -sc)
                for b in range(2):
                    nc.vector.tensor_scalar_mul(bsl(xt, b), bsl(xt, b), sc)
                for b in range(2):
                    ps = eps.tile([P, 4 * P], FP, space="PSUM", name="ps", tag="eps")[:, :P]
                    nc.tensor.matmul(ps[:], lhsT=bsl(nxt, b), rhs=bsl(xt, b), start=True, stop=True)
                    nc.vector.tensor_copy(out=bsl(x2, b), in_=ps[:])
                for b in range(2):
                    ps = eps.tile([P, 4 * P], FP, space="PSUM", name="ps", tag="eps")[:, :P]
                    nc.tensor.matmul(ps[:], lhsT=bsl(x2, b), rhs=bsl(xt, b), start=True, stop=True)
                    nc.vector.tensor_copy(out=bsl(x3, b), in_=ps[:])
                for b in range(2):
                    ps = eps.tile([P, 4 * P], FP, space="PSUM", name="ps", tag="eps")[:, :P]
                    nc.tensor.matmul(ps[:], lhsT=bsl(x2, b), rhs=bsl(x2, b), start=True, stop=True)
                    nc.vector.tensor_copy(out=bsl(x4, b), in_=ps[:])

                # B_i = c0 I + c1 X + c2 X2 + c3 X3 ; Bm_i with -X, -X3
                bp = [[None] * 2 for _ in range(4)]
                bm = [[None] * 2 for _ in range(4)]
                for i in range(4):
                    c0, c1, c2, c3 = _C[4 * i : 4 * i + 4]
                    ic = ep.tile([P, P], FP, name=f"ic{i}")
                    nc.vector.tensor_scalar_mul(ic[:], ident[:], c0)
                    for b in range(2):
                        tmp = ep.tile([P, P], FP, name=f"tmp{i}{b}")
                        nc.vector.tensor_scalar_mul(tmp[:], bsl(xt, b), c1)
                        vo = ep.tile([P, P], FP, name=f"vo{i}{b}")
                        nc.vector.scalar_tensor_tensor(
                            out=vo[:], in0=bsl(x3, b), scalar=c3, in1=tmp[:],
                            op0=MULT, op1=ADD,
                        )
                        ve = ep.tile([P, P], FP, name=f"ve{i}{b}")
                        nc.vector.scalar_tensor_tensor(
                            out=ve[:], in0=bsl(x2, b), scalar=c2, in1=ic[:],
                            op0=MULT, op1=ADD,
                        )
                        tb_ = ep.tile([P, P], FP, name=f"tb{i}{b}")
                        nc.vector.tensor_tensor(out=tb_[:], in0=ve[:], in1=vo[:], op=ADD)
                        tm_ = ep.tile([P, P], FP, name=f"tm{i}{b}")
                        nc.vector.tensor_tensor(out=tm_[:], in0=ve[:], in1=vo[:], op=SUB)
                        bp[i][b] = tb_
                        bm[i][b] = tm_

                # Paterson-Stockmeyer chains -> S = e^X, ST = e^-X (scaled)
                s_t, st_t = [None, None], [None, None]
                for b in range(2):
                    for which, bs, dst in ((0, bp, s_t), (1, bm, st_t)):
                        z = bs[3][b]
                        for i in (2, 1, 0):
                            ps = eps.tile([P, 4 * P], FP, space="PSUM", name="psz", tag="eps")[:, :P]
                            nc.tensor.matmul(ps[:], lhsT=bsl(x4, b), rhs=z[:], start=True, stop=True)
                            znew = ep.tile([P, P], FP, name=f"z{which}{b}{i}")
                            nc.vector.tensor_tensor(out=znew[:], in0=ps[:], in1=bs[i][b][:], op=ADD)
                            z = znew
                        dst[b] = z

                for lvl in range(S_SCALE):
                    for b in range(2):
                        ps1 = eps.tile([P, 4 * P], FP, space="PSUM", name="psq1", tag="eps")[:, :P]
                        nc.tensor.matmul(ps1[:], lhsT=st_t[b][:], rhs=s_t[b][:], start=True, stop=True)
                        ps2 = eps.tile([P, 4 * P], FP, space="PSUM", name="psq2", tag="eps")[:, :P]
                        nc.tensor.matmul(ps2[:], lhsT=s_t[b][:], rhs=st_t[b][:], start=True, stop=True)
                        sn = ep.tile([P, P], FP, name=f"s{lvl}{b}")
                        stn = ep.tile([P, P], FP, name=f"st{lvl}{b}")
                        nc.vector.tensor_copy(out=sn[:], in_=ps1[:])
                        nc.vector.tensor_copy(out=stn[:], in_=ps2[:])
                        s_t[b], st_t[b] = sn, stn

                # final: E = expP (S), tables level 1 (hi/lo bf16 pairs)
                def slot(t, e):
                    return t[:, e * P : (e + 1) * P]

                def wpair(th, tl, dst_h, dst_l, src):
                    nc.scalar.copy(out=dst_h, in_=src)
                    nc.vector.tensor_tensor(out=dst_l, in0=src, in1=dst_h, op=SUB)

                def wpair_e(th, tl, e, src):
                    wpair(th, tl, slot(th, e), slot(tl, e), src)

                for b in range(2):
                    wpair(eh, el, bsl(eh, b), bsl(el, b), s_t[b][:])
                    wpair_e(ftab_h, ftab_l, TB[1] + b, st_t[b][:])
                    wpair_e(tabt_h, tabt_l, TB[1] + b, s_t[b][:])
                wpair_e(ftab_h, ftab_l, 0, ident[:])
                wpair_e(tabt_h, tabt_l, 0, ident[:])

                # ---------------- tables j=2..5 ----------------
                for j in range(2, NLEVELS + 1):
                    half = 1 << (j - 1)
                    # plain: F_j[2t+b] = M_b @ F_{j-1}[t] ; out = E_b^T... lhsT=E_b
                    for b in range(2):
                        for cs in range(0, half, 4):
                            cn = min(4, half - cs)
                            ps = eps.tile([P, 4 * P], FP, space="PSUM", name="pstab", tag="eps")
                            rsl = slice((TB[j - 1] + cs) * P, (TB[j - 1] + cs + cn) * P)
                            nc.tensor.matmul(
                                ps[:, : cn * P], lhsT=bsl(eh, b), rhs=ftab_h[:, rsl],
                                start=True, stop=False,
                            )
                            nc.tensor.matmul(
                                ps[:, : cn * P], lhsT=bsl(eh, b), rhs=ftab_l[:, rsl],
                                start=False, stop=False,
                            )
                            nc.tensor.matmul(
                                ps[:, : cn * P], lhsT=bsl(el, b), rhs=ftab_h[:, rsl],
                                start=False, stop=True,
                            )
                            # interleaved write: entries TB[j] + 2*(cs+t) + b,
                            # one strided hi + one strided lo op per chunk
                            e0 = TB[j] + 2 * cs + b
                            dst_h = ftab_h3[:, e0 : e0 + 2 * cn - 1 : 2, :]
                            dst_l = ftab_l3[:, e0 : e0 + 2 * cn - 1 : 2, :]
                            src3 = ps[:, : cn * P].rearrange("p (t c) -> p t c", c=P)
                            nc.scalar.copy(out=dst_h, in_=src3)
                            nc.vector.tensor_tensor(out=dst_l, in0=src3, in1=dst_h, op=SUB)
                    # transposed: TABT[TB_j + 2t + b] = F_{j-1}[t]^T @ expP_b
                    # (only pairs with a used entry; pair written in one op)
                    for t in range(half):
                        if used_t is not None and not (
                            TB[j] + 2 * t in used_t or TB[j] + 2 * t + 1 in used_t
                        ):
                            continue
                        ps = eps.tile([P, 4 * P], FP, space="PSUM", name="pstt", tag="eps")[:, :2 * P]
                        nc.tensor.matmul(
                            ps[:], lhsT=slot(ftab_h, TB[j - 1] + t), rhs=eh[:, 0 : 2 * P],
                            start=True, stop=False,
                        )
                        nc.tensor.matmul(
                            ps[:], lhsT=slot(ftab_h, TB[j - 1] + t), rhs=el[:, 0 : 2 * P],
                            start=False, stop=False,
                        )
                        nc.tensor.matmul(
                            ps[:], lhsT=slot(ftab_l, TB[j - 1] + t), rhs=eh[:, 0 : 2 * P],
                            start=False, stop=True,
                        )
                        dph = tabt_h[:, (TB[j] + 2 * t) * P : (TB[j] + 2 * t + 2) * P]
                        dpl = tabt_l[:, (TB[j] + 2 * t) * P : (TB[j] + 2 * t + 2) * P]
                        nc.scalar.copy(out=dph, in_=ps[:])
                        nc.vector.tensor_tensor(out=dpl, in0=ps[:], in1=dph, op=SUB)

            # ---------------- per-position stage ----------------
            with (
                tc.tile_pool(name="ps1", bufs=5, space="PSUM") as ps1p,
                tc.tile_pool(name="ps2", bufs=3, space="PSUM") as ps2p,
                tc.tile_pool(name="outp", bufs=8) as outp,
            ):
                k = 0
                for e, cap in groups1:
                    lhs_h = tabt_h[:, e * P : (e + 1) * P]
                    lhs_l = tabt_l[:, e * P : (e + 1) * P]
                    for _ in range(cap):
                        rv = nc.values_load(
                            o1r[0:1, k : k + 1], engines={PE},
                            min_val=0, max_val=(NTAB - 1) * P,
                            skip_runtime_bounds_check=True,
                        )
                        ps = ps1p.tile([P, P], FP, space="PSUM", name="ps1t")
                        nc.tensor.matmul(
                            ps[:], lhsT=lhs_h, rhs=ftab_h[:, bass.ds(rv, P)],
                            start=True, stop=False,
                        )
                        nc.tensor.matmul(
                            ps[:], lhsT=lhs_h, rhs=ftab_l[:, bass.ds(rv, P)],
                            start=False, stop=False,
                        )
                        nc.tensor.matmul(
                            ps[:], lhsT=lhs_l, rhs=ftab_h[:, bass.ds(rv, P)],
                            start=False, stop=True,
                        )
                        rwa = nc.values_load(
                            o1w[0:1, k : k + 1], engines={mybir.EngineType.Activation},
                            min_val=0, max_val=S2 * P,
                            skip_runtime_bounds_check=True,
                        )
                        rwv = nc.values_load(
                            o1w[0:1, k : k + 1], engines={DVE},
                            min_val=0, max_val=S2 * P,
                            skip_runtime_bounds_check=True,
                        )
                        nc.scalar.copy(out=wstage_h[:, bass.ds(rwa, P)], in_=ps[:])
                        nc.vector.tensor_tensor(
                            out=wstage_l[:, bass.ds(rwv, P)], in0=ps[:],
                            in1=wstage_h[:, bass.ds(rwv, P)], op=SUB,
                        )
                        k += 1

                # step2: static wide matmuls over 4-lane chunks (lane
                # numbering is group-padded to x4; wstage pre-zeroed so the
                # full 4-lane width is always valid)
                k = 0
                for e, cap in groups2:
                    lhs_h = tabt_h[:, e * P : (e + 1) * P]
                    lhs_l = tabt_l[:, e * P : (e + 1) * P]
                    capp = ((cap + 3) // 4) * 4
                    for cs in range(0, capp, 4):
                        lane0 = k + cs
                        rsl_h = wstage_h[:, lane0 * P : (lane0 + 4) * P]
                        rsl_l = wstage_l[:, lane0 * P : (lane0 + 4) * P]
                        ps = ps2p.tile([P, 4 * P], FP, space="PSUM", name="ps2t")
                        nc.tensor.matmul(ps[:], lhsT=lhs_h, rhs=rsl_h, start=True, stop=False)
                        nc.tensor.matmul(ps[:], lhsT=lhs_h, rhs=rsl_l, start=False, stop=False)
                        nc.tensor.matmul(ps[:], lhsT=lhs_l, rhs=rsl_h, start=False, stop=True)
                        ch = outp.tile([P, 4 * P], FP, name="outchunk")
                        if (lane0 // 4) % 3 != 2:
                            nc.vector.tensor_copy(out=ch[:], in_=ps[:])
                        else:
                            nc.scalar.copy(out=ch[:], in_=ps[:])
                        nc.sync.dma_start(out=raw_d[lane0 // 4], in_=ch[:])
                    k += capp
    return nc


def _run(nc, in_maps, trace=False):
    return run_bass_kernel_spmd(nc, in_maps, list(range(NCORES)), trace=trace)


def build_all(unique, primitives):
    """Build program + per-core inputs. Returns (nc, in_maps, assemble)."""
    unique = np.asarray(unique).astype(np.int64)
    primitives = np.ascontiguousarray(np.asarray(primitives, dtype=np.float32))
    groups1, groups2, S1, S2P, o1r, o1w, outmap, used_t = _plan(unique)

    nc = bacc.Bacc("TRN2", target_bir_lowering=False, debug=False)
    _build(nc, groups1, groups2, S1, S2P, used_t)
    nc.compile()

    in_maps = [
        {
            "prims": primitives,
            "offs1_rhs": o1r[c],
            "offs1_wdst": o1w[c],
        }
        for c in range(NCORES)
    ]

    def assemble(results):
        full = np.empty((NCORES * NPOS, P, P), np.float32)
        for c in range(NCORES):
            raw = results[c]["rawout"]  # [nch, P, 4P]
            for lane, g in outmap[c].items():
                full[g] = raw[lane // 4][:, (lane % 4) * P : (lane % 4 + 1) * P]
        return full

    return nc, in_maps, assemble


def kernel(unique, primitives):
    nc, in_maps, assemble = build_all(unique, primitives)
    res = _run(nc, in_maps, trace=False)
    return assemble(res.results)
